# revision 35
# baseline (speedup 1.0000x reference)
"""Trainium2 Bass kernel for nn_CombinatorialClassifier.

Computation (reference):
    logits = einsum('bf,pqf->bpq', x, W) + b        # [B,P,Q]
    logp   = log_softmax(logits, axis=2)            # [B,P,Q]
    out    = take_along_axis(logp, part_idx, 2)     # [B,P,C]

Shapes: B=256, P=64, Q=128, C=1000, F=2048.

Sharding: expert-parallel over P across 8 cores (8 partitionings per
core).  Each core reads the full x and its W/b/part_idx slice and
writes its disjoint [B, 8, C] slice of the output.  No collectives.

Per-core dataflow ("orientation A" — q lives on SBUF partitions):
  - main matmul:   psum_lin[q, b] += WT_k[f,q].T @ xT_k[f,b], bias
    folded in as a K=1 accumulate matmul (bias[q] x ones[b]).
  - sumexp over q: ones[128,1].T @ exp[q,b] matmul (PE reduces over
    partitions), lse = Ln(sumexp) on ScalarE.
  - gather+logsoftmax in one PSUM group:
        psum_out[b, c] = linT[q,b].T @ OH[q,c] + lse[b].T @ (-1)[c]
    (the K=1 lse matmul also transposes lse into the partition dim).
    OH_p[q, c] = (q == part_idx[p,c]) is built per-p on DVE with an
    is_equal against a partition iota.

This walrus build only accepts ONE sync-wait command per compute/DMA
instruction, which dictates most of the structure:
  - x|W share one DMA per k-tile ("xw"); bias|ones share one DMA
    ("bo"), so each matmul joins on a single semaphore.
  - every SBUF tile is used exactly once (fresh slot) -> no
    WAR/WAW slot-release waits anywhere.
  - idx and iota for partitioning p are DMAd back-to-back so the
    SWDGE round-robin lands them on the same queue semaphore; the
    is_equal TT then joins on that one sem.
  - PSUM->SBUF result copies all run on DVE into per-(p-pair,bt)
    group tiles; each output DMA (on the ACT HWDGE) is preceded by a
    tiny ACT "observer" op that absorbs the DVE producer wait, so the
    DMA itself only carries its queue-predecessor wait.
  - bf16 for x/W (also halves their HBM traffic); the gather path is
    float32r (full-rate PE fp32).
"""

import numpy as np

B, P, Q, C, F = 256, 64, 128, 1000, 2048
NCORES = 8
PL = P // NCORES          # partitionings per core
KT = F // 128             # contraction tiles
BT = B // 128             # batch tiles for the gather matmul
C_CHUNKS = [(0, 512), (512, C - 512)]

MAIN_BF16 = True          # store/stream x,W as bf16 and matmul in bf16
GATHER_R = True           # gather/lse/sumexp matmul operands in float32r


def _build_nc():
    import concourse.bass as bass
    import concourse.tile as tile
    from concourse import mybir
    from contextlib import ExitStack

    DT = mybir.dt.float32
    HT = mybir.dt.float16
    MDT = mybir.dt.bfloat16 if MAIN_BF16 else mybir.dt.float32r
    GDT = mybir.dt.float32r if GATHER_R else DT

    nc = bass.Bass()
    xw_d = nc.declare_dram_parameter("xw", [KT, 128, B + PL * Q], MDT,
                                     isOutput=False)
    bo_d = nc.declare_dram_parameter("bo", [1, PL * Q + B], MDT,
                                     isOutput=False)
    # idxq[q, p, :C] = part_idx[p, :] (same on every partition row) and
    # idxq[q, p, C] = q — idx and iota in ONE tensor/DMA, so the
    # is_equal TT joins on a single DMA semaphore
    idx_d = nc.declare_dram_parameter("idxq", [Q, PL, C + 1], HT,
                                      isOutput=False)
    out_d = nc.declare_dram_parameter("out", [B, PL, C], DT, isOutput=True)

    with ExitStack() as ctx:
        tc = ctx.enter_context(tile.TileContext(nc))
        singles = ctx.enter_context(tc.tile_pool(name="singles", bufs=1))
        ps_lin = ctx.enter_context(
            tc.tile_pool(name="ps_lin", bufs=1, space=bass.MemorySpace.PSUM))
        ps_sum = ctx.enter_context(
            tc.tile_pool(name="ps_sum", bufs=2, space=bass.MemorySpace.PSUM))
        ps_out = ctx.enter_context(
            tc.tile_pool(name="ps_out", bufs=2, space=bass.MemorySpace.PSUM))

        def fresh(shape, dtype, tag):
            return singles.tile(shape, dtype, tag=tag, name=tag)

        # ---- static tiles (all fresh, single-use) -------------------
        # bo first: the bias matmul opens every PSUM accumulation group,
        # so its DMA must land before the first xw tile.  idxq early so
        # the one-hot build overlaps the p=0 main matmuls.
        bo_sb = fresh([1, PL * Q + B], MDT, "bo")
        nc.sync.dma_start(out=bo_sb[:], in_=bo_d[:])
        xwk = []
        idx_sb = None
        for k in range(KT):
            t = fresh([128, B + PL * Q], MDT, f"xwk{k}")
            nc.sync.dma_start(out=t[:], in_=xw_d[k])
            xwk.append(t)
            if k == 3:
                idx_sb = fresh([128, PL, C + 1], HT, "idxq")
                nc.sync.dma_start(out=idx_sb[:], in_=idx_d[:])

        # ACT-produced constants so the ACT-side matmuls join on ACT
        ones_col = fresh([128, 1], GDT, "ones")
        nc.scalar.activation(out=ones_col[:], in_=xwk[0][:, 0:1],
                             func=mybir.ActivationFunctionType.Copy,
                             bias=1.0, scale=0.0)
        negones_sb = fresh([1, 512], GDT, "negones")
        nc.scalar.activation(out=negones_sb[:], in_=bo_sb[0:1, 0:512],
                             func=mybir.ActivationFunctionType.Copy,
                             bias=-1.0, scale=0.0)

        obs_scratch = fresh([1, 4 * PL], DT, "obs")

        # ---- main matmuls, k-outer ----------------------------------
        # One big 4-bank PSUM holds all 8 partitionings; each arriving
        # k-tile feeds 8 dense back-to-back matmuls so the PE stays
        # saturated (and HAM-warm) while the xw stream lands.
        psum_big = ps_lin.tile([128, PL, B], DT)
        for p in range(PL):
            nc.tensor.matmul(
                psum_big[:, p, :],
                bo_sb[:, p * Q:(p + 1) * Q],
                bo_sb[:, PL * Q:],
                start=True, stop=False, skip_group_check=True)
        for k in range(KT):
            for p in range(PL):
                nc.tensor.matmul(
                    psum_big[:, p, :],
                    xwk[k][:, B + p * Q:B + (p + 1) * Q],
                    xwk[k][:, :B],
                    start=False,
                    stop=(k == KT - 1),
                    skip_group_check=True,
                )

        # ---- per-partitioning softmax + gather pipeline -------------
        og_tiles = {}
        n_obs = 0
        for p in range(PL):
            psum_lin = psum_big[:, p, :]
            # one-hot build for this p on DVE (single DMA sem join)
            oh_p = fresh([128, C], GDT, f"oh{p}")
            nc.vector.tensor_tensor(
                out=oh_p[:],
                in0=idx_sb[:, p, :C],
                in1=idx_sb[:, p, C:C + 1].broadcast_to((128, C)),
                op=mybir.AluOpType.is_equal,
            )

            linT = fresh([128, B], GDT, f"lin{p}")
            nc.vector.tensor_copy(linT[:], psum_lin[:])
            expT = fresh([128, B], GDT, f"exp{p}")
            nc.scalar.activation(
                out=expT[:], in_=linT[:],
                func=mybir.ActivationFunctionType.Exp)

            psum_sum = ps_sum.tile([1, B], DT)
            nc.tensor.matmul(
                psum_sum[:], ones_col[:], expT[:],
                start=True, stop=True)
            lse = fresh([1, B], GDT, f"lse{p}")
            nc.scalar.activation(
                out=lse[:], in_=psum_sum[:],
                func=mybir.ActivationFunctionType.Ln)

            pair = p // 2
            for bt in range(BT):
                bsl = slice(bt * 128, (bt + 1) * 128)
                if p % 2 == 0:
                    og_new = fresh([128, 2, C], DT, f"og{pair}_{bt}")
                    og_tiles[(pair, bt)] = og_new
                og = og_tiles[(pair, bt)]
                last_copy = None
                for (c0, cw) in C_CHUNKS:
                    psum_out = ps_out.tile([128, 512], DT)
                    nc.tensor.matmul(
                        psum_out[:, :cw],
                        linT[:, bsl],
                        oh_p[:, c0:c0 + cw],
                        start=True, stop=False)
                    nc.tensor.matmul(
                        psum_out[:, :cw],
                        lse[:, bsl],
                        negones_sb[:, :cw],
                        start=False, stop=True)
                    last_copy = nc.vector.tensor_copy(
                        og[:, p % 2, c0:c0 + cw], psum_out[:, :cw])
                if p % 2 == 1:
                    # ACT observer absorbs the DVE producer wait; the
                    # DMA then only carries its queue-predecessor wait
                    obs = nc.scalar.activation(
                        out=obs_scratch[0:1, n_obs:n_obs + 1],
                        in_=og[0:1, 1, C - 1:C],
                        func=mybir.ActivationFunctionType.Copy,
                        bias=0.0, scale=1.0)
                    n_obs += 1
                    dma = nc.scalar.dma_start(
                        out=out_d[bsl, p - 1:p + 1, :],
                        in_=og[:])
                    tile.add_dep_helper(dma.ins, obs.ins, sync=False,
                                        reason="dma after observer")

    _install_drain_split(nc)
    return nc


def _install_drain_split(nc, chunk=1):
    """The kernel-tail Drain waits on every live semaphore (~11), but
    this walrus build's CTRL_NO encoding fits only a couple of sync
    commands.  Splitting the drain into a chain of drains, each
    carrying `chunk` waits, is semantically identical (sequential SP
    sem waits).  Patch at serialization time so every consumer of
    nc.to_json_bytes() sees the legal form."""
    import copy
    import json

    orig = nc.to_json_bytes

    def patched():
        m = json.loads(orig())
        for fn in m["functions"]:
            for bb in fn["blocks"]:
                out = []
                for inst in bb["instructions"]:
                    si = inst.get("sync_info")
                    if (inst.get("opcode") == "Drain" and si
                            and si.get("on_wait")
                            and len(si["on_wait"]) > chunk):
                        waits = si["on_wait"]
                        head, keep = waits[:-chunk], waits[-chunk:]
                        for j in range(0, len(head), chunk):
                            clone = copy.deepcopy(inst)
                            clone["name"] = f"{inst['name']}-ds{j}"
                            clone["sync_info"] = {
                                "on_wait": head[j:j + chunk],
                                "on_update": [],
                            }
                            out.append(clone)
                        si["on_wait"] = keep
                    out.append(inst)
                bb["instructions"] = out
        return json.dumps(m).encode()

    nc.to_json_bytes = patched


def _host_inputs(x, W, b, part_idx):
    """Build the 8 per-core input maps."""
    import ml_dtypes

    mm_np = ml_dtypes.bfloat16 if MAIN_BF16 else np.float32
    xT = x.T.reshape(KT, 128, B).astype(mm_np)                # [KT,128,B]
    in_maps = []
    for i in range(NCORES):
        sl = slice(i * PL, (i + 1) * PL)
        WT = W[sl].transpose(2, 0, 1).reshape(
            KT, 128, PL * Q).astype(mm_np)                    # [KT,128,PL*Q]
        xw = np.empty((KT, 128, B + PL * Q), dtype=mm_np)
        xw[:, :, :B] = xT
        xw[:, :, B:] = WT
        bo = np.empty((1, PL * Q + B), dtype=mm_np)
        bo[0, :PL * Q] = b[sl].reshape(-1)
        bo[0, PL * Q:] = 1.0
        idxq = np.empty((Q, PL, C + 1), dtype=np.float16)
        idxq[:, :, :C] = part_idx[sl].astype(np.float16)[None, :, :]
        idxq[:, :, C] = np.arange(Q, dtype=np.float16)[:, None]
        in_maps.append({"xw": xw, "bo": bo, "idxq": idxq})
    return in_maps


def kernel(x, W, b, part_idx, _trace=False):
    from concourse.bass_utils import run_bass_kernel_spmd

    x = np.asarray(x, dtype=np.float32)
    W = np.asarray(W, dtype=np.float32)
    b = np.asarray(b, dtype=np.float32)
    part_idx = np.asarray(part_idx)

    nc = _build_nc()
    in_maps = _host_inputs(x, W, b, part_idx)
    res = run_bass_kernel_spmd(nc, in_maps, list(range(NCORES)),
                               trace=_trace)
    out = np.concatenate([r["out"] for r in res.results], axis=1)
    if _trace:
        return out, res
    return out


# revision 39
# speedup vs baseline: 1.3475x; 1.3475x over previous
"""Trainium2 Bass kernel for nn_CombinatorialClassifier.

Computation (reference):
    logits = einsum('bf,pqf->bpq', x, W) + b        # [B,P,Q]
    logp   = log_softmax(logits, axis=2)            # [B,P,Q]
    out    = take_along_axis(logp, part_idx, 2)     # [B,P,C]

Shapes: B=256, P=64, Q=128, C=1000, F=2048.

Sharding: expert-parallel over P across 8 cores (8 partitionings per
core).  Each core reads the full x and its W/b/part_idx slice and
writes its disjoint [B, 8, C] slice of the output.  No collectives.

Per-core dataflow ("orientation A" — q lives on SBUF partitions):
  - main matmul:   psum_lin[q, b] += WT_k[f,q].T @ xT_k[f,b], bias
    folded in as a K=1 accumulate matmul (bias[q] x ones[b]).
  - sumexp over q: ones[128,1].T @ exp[q,b] matmul (PE reduces over
    partitions), lse = Ln(sumexp) on ScalarE.
  - gather+logsoftmax in one PSUM group:
        psum_out[b, c] = linT[q,b].T @ OH[q,c] + lse[b].T @ (-1)[c]
    (the K=1 lse matmul also transposes lse into the partition dim).
    OH_p[q, c] = (q == part_idx[p,c]) is built per-p on DVE with an
    is_equal against a partition iota.

This walrus build only accepts ONE sync-wait command per compute/DMA
instruction, which dictates most of the structure:
  - x|W share one DMA per k-tile ("xw"); bias|ones share one DMA
    ("bo"), so each matmul joins on a single semaphore.
  - every SBUF tile is used exactly once (fresh slot) -> no
    WAR/WAW slot-release waits anywhere.
  - idx and iota for partitioning p are DMAd back-to-back so the
    SWDGE round-robin lands them on the same queue semaphore; the
    is_equal TT then joins on that one sem.
  - PSUM->SBUF result copies all run on DVE into per-(p-pair,bt)
    group tiles; each output DMA (on the ACT HWDGE) is preceded by a
    tiny ACT "observer" op that absorbs the DVE producer wait, so the
    DMA itself only carries its queue-predecessor wait.
  - bf16 for x/W (also halves their HBM traffic); the gather path is
    float32r (full-rate PE fp32).
"""

import numpy as np

B, P, Q, C, F = 256, 64, 128, 1000, 2048
NCORES = 8
PL = P // NCORES          # partitionings per core
KT = F // 128             # contraction tiles
BT = B // 128             # batch tiles for the gather matmul
C_CHUNKS = [(0, 512), (512, C - 512)]

MAIN_BF16 = True          # store/stream x,W as bf16 and matmul in bf16
GATHER_R = True           # gather/lse/sumexp matmul operands in float32r


def _build_nc():
    import concourse.bass as bass
    import concourse.tile as tile
    from concourse import mybir
    from contextlib import ExitStack

    DT = mybir.dt.float32
    HT = mybir.dt.float16
    MDT = mybir.dt.bfloat16 if MAIN_BF16 else mybir.dt.float32r
    # bf16 gather operands: 2-byte stationary loads keep the PE at full
    # rate (fp32r 4-byte weight loads measured ~2x slower per matmul)
    GDT = mybir.dt.bfloat16

    nc = bass.Bass()
    xw_d = nc.declare_dram_parameter("xw", [KT, 128, B + PL * Q], MDT,
                                     isOutput=False)
    bo_d = nc.declare_dram_parameter("bo", [1, PL * Q + B], MDT,
                                     isOutput=False)
    # idxq[q, p, :C] = part_idx[p, :] (same on every partition row) and
    # idxq[q, p, C] = q — idx and iota in ONE tensor/DMA, so the
    # is_equal TT joins on a single DMA semaphore
    idx_d = nc.declare_dram_parameter("idxq", [Q, PL, C + 1], HT,
                                      isOutput=False)
    out_d = nc.declare_dram_parameter("out", [B, PL, C], DT, isOutput=True)

    with ExitStack() as ctx:
        tc = ctx.enter_context(tile.TileContext(nc))
        singles = ctx.enter_context(tc.tile_pool(name="singles", bufs=1))
        ps_lin = ctx.enter_context(
            tc.tile_pool(name="ps_lin", bufs=2, space=bass.MemorySpace.PSUM))
        ps_sum = ctx.enter_context(
            tc.tile_pool(name="ps_sum", bufs=2, space=bass.MemorySpace.PSUM))
        ps_out = ctx.enter_context(
            tc.tile_pool(name="ps_out", bufs=4, space=bass.MemorySpace.PSUM))

        def fresh(shape, dtype, tag):
            return singles.tile(shape, dtype, tag=tag, name=tag)

        # ---- static tiles (all fresh, single-use) -------------------
        xwk = []
        for k in range(KT):
            t = fresh([128, B + PL * Q], MDT, f"xwk{k}")
            nc.sync.dma_start(out=t[:], in_=xw_d[k])
            xwk.append(t)
        bo_sb = fresh([1, PL * Q + B], MDT, "bo")
        nc.sync.dma_start(out=bo_sb[:], in_=bo_d[:])
        idx_sb = fresh([128, PL, C + 1], HT, "idxq")
        nc.sync.dma_start(out=idx_sb[:], in_=idx_d[:])

        # ACT-produced constants so the ACT-side matmuls join on ACT
        ones_col = fresh([128, 1], GDT, "ones")
        nc.scalar.activation(out=ones_col[:], in_=xwk[0][:, 0:1],
                             func=mybir.ActivationFunctionType.Copy,
                             bias=1.0, scale=0.0)
        negones_sb = fresh([1, 512], GDT, "negones")
        nc.scalar.activation(out=negones_sb[:], in_=bo_sb[0:1, 0:512],
                             func=mybir.ActivationFunctionType.Copy,
                             bias=-1.0, scale=0.0)

        obs_scratch = fresh([1, 4 * PL], DT, "obs")

        # ---- per-partitioning pipeline ------------------------------
        og_tiles = {}
        n_obs = 0
        for p in range(PL):
            psum_lin = ps_lin.tile([128, B], DT)
            # bias: K=1 matmul bias[q] x ones[b] opens the accumulation
            nc.tensor.matmul(
                psum_lin[:],
                bo_sb[:, p * Q:(p + 1) * Q],
                bo_sb[:, PL * Q:],
                start=True, stop=False)
            for k in range(KT):
                nc.tensor.matmul(
                    psum_lin[:],
                    xwk[k][:, B + p * Q:B + (p + 1) * Q],
                    xwk[k][:, :B],
                    start=False,
                    stop=(k == KT - 1),
                )

            # one-hot build for this p on DVE (single DMA sem join)
            oh_p = fresh([128, C], GDT, f"oh{p}")
            nc.vector.tensor_tensor(
                out=oh_p[:],
                in0=idx_sb[:, p, :C],
                in1=idx_sb[:, p, C:C + 1].broadcast_to((128, C)),
                op=mybir.AluOpType.is_equal,
            )

            linT = fresh([128, B], GDT, f"lin{p}")
            nc.vector.tensor_copy(linT[:], psum_lin[:])
            expT = fresh([128, B], GDT, f"exp{p}")
            nc.scalar.activation(
                out=expT[:], in_=linT[:],
                func=mybir.ActivationFunctionType.Exp)

            psum_sum = ps_sum.tile([1, B], DT)
            nc.tensor.matmul(
                psum_sum[:], ones_col[:], expT[:],
                start=True, stop=True)
            lse = fresh([1, B], GDT, f"lse{p}")
            nc.scalar.activation(
                out=lse[:], in_=psum_sum[:],
                func=mybir.ActivationFunctionType.Ln)

            pair = p // 2
            for bt in range(BT):
                bsl = slice(bt * 128, (bt + 1) * 128)
                if p % 2 == 0:
                    og_new = fresh([128, 2, C], DT, f"og{pair}_{bt}")
                    og_tiles[(pair, bt)] = og_new
                og = og_tiles[(pair, bt)]
                last_copy = None
                for (c0, cw) in C_CHUNKS:
                    psum_out = ps_out.tile([128, 512], DT)
                    nc.tensor.matmul(
                        psum_out[:, :cw],
                        linT[:, bsl],
                        oh_p[:, c0:c0 + cw],
                        start=True, stop=False)
                    nc.tensor.matmul(
                        psum_out[:, :cw],
                        lse[:, bsl],
                        negones_sb[:, :cw],
                        start=False, stop=True)
                    last_copy = nc.vector.tensor_copy(
                        og[:, p % 2, c0:c0 + cw], psum_out[:, :cw])
                if p % 2 == 1:
                    # ACT observer absorbs the DVE producer wait; the
                    # DMA then only carries its queue-predecessor wait
                    obs = nc.scalar.activation(
                        out=obs_scratch[0:1, n_obs:n_obs + 1],
                        in_=og[0:1, 1, C - 1:C],
                        func=mybir.ActivationFunctionType.Copy,
                        bias=0.0, scale=1.0)
                    n_obs += 1
                    dma = nc.scalar.dma_start(
                        out=out_d[bsl, p - 1:p + 1, :],
                        in_=og[:])
                    tile.add_dep_helper(dma.ins, obs.ins, sync=False,
                                        reason="dma after observer")

    _install_drain_split(nc)
    return nc


def _install_drain_split(nc, chunk=1):
    """The kernel-tail Drain waits on every live semaphore (~11), but
    this walrus build's CTRL_NO encoding fits only a couple of sync
    commands.  Splitting the drain into a chain of drains, each
    carrying `chunk` waits, is semantically identical (sequential SP
    sem waits).  Patch at serialization time so every consumer of
    nc.to_json_bytes() sees the legal form."""
    import copy
    import json

    orig = nc.to_json_bytes

    def patched():
        m = json.loads(orig())
        for fn in m["functions"]:
            for bb in fn["blocks"]:
                out = []
                for inst in bb["instructions"]:
                    si = inst.get("sync_info")
                    if (inst.get("opcode") == "Drain" and si
                            and si.get("on_wait")
                            and len(si["on_wait"]) > chunk):
                        waits = si["on_wait"]
                        head, keep = waits[:-chunk], waits[-chunk:]
                        for j in range(0, len(head), chunk):
                            clone = copy.deepcopy(inst)
                            clone["name"] = f"{inst['name']}-ds{j}"
                            clone["sync_info"] = {
                                "on_wait": head[j:j + chunk],
                                "on_update": [],
                            }
                            out.append(clone)
                        si["on_wait"] = keep
                    out.append(inst)
                bb["instructions"] = out
        return json.dumps(m).encode()

    nc.to_json_bytes = patched


def _host_inputs(x, W, b, part_idx):
    """Build the 8 per-core input maps."""
    import ml_dtypes

    mm_np = ml_dtypes.bfloat16 if MAIN_BF16 else np.float32
    xT = x.T.reshape(KT, 128, B).astype(mm_np)                # [KT,128,B]
    in_maps = []
    for i in range(NCORES):
        sl = slice(i * PL, (i + 1) * PL)
        WT = W[sl].transpose(2, 0, 1).reshape(
            KT, 128, PL * Q).astype(mm_np)                    # [KT,128,PL*Q]
        xw = np.empty((KT, 128, B + PL * Q), dtype=mm_np)
        xw[:, :, :B] = xT
        xw[:, :, B:] = WT
        bo = np.empty((1, PL * Q + B), dtype=mm_np)
        bo[0, :PL * Q] = b[sl].reshape(-1)
        bo[0, PL * Q:] = 1.0
        idxq = np.empty((Q, PL, C + 1), dtype=np.float16)
        idxq[:, :, :C] = part_idx[sl].astype(np.float16)[None, :, :]
        idxq[:, :, C] = np.arange(Q, dtype=np.float16)[:, None]
        in_maps.append({"xw": xw, "bo": bo, "idxq": idxq})
    return in_maps


def kernel(x, W, b, part_idx, _trace=False):
    from concourse.bass_utils import run_bass_kernel_spmd

    x = np.asarray(x, dtype=np.float32)
    W = np.asarray(W, dtype=np.float32)
    b = np.asarray(b, dtype=np.float32)
    part_idx = np.asarray(part_idx)

    nc = _build_nc()
    in_maps = _host_inputs(x, W, b, part_idx)
    res = run_bass_kernel_spmd(nc, in_maps, list(range(NCORES)),
                               trace=_trace)
    out = np.concatenate([r["out"] for r in res.results], axis=1)
    if _trace:
        return out, res
    return out
